# revision 25
# baseline (speedup 1.0000x reference)
"""Trainium2 Bass kernel for nn_CrossAttention (self-attention, B=2, S=2048,
16 heads x 64 dim, d_model=1024).

Sharding: batch*heads across 8 cores -> each core owns 2 heads for both
batches. Each core receives the full (pre-transposed, fp16) hidden states and
its 128-column slice of w_q/w_k/w_v (w_q pre-scaled by 1/sqrt(64)) plus its
128-row slice of w_o. Cores emit fp16 partial outputs [4096, 1024] (the w_o
contraction over the core's 128 inner dims); the host sums the partials in
fp32 and adds b_o.

Per-core dataflow (all matmuls fp16 operands, fp32 PSUM accumulation):
  1. Q^T/K^T/V^T [128, 4096] = w.T @ hs^T   (feature-major projections)
  2. V^T -> V plain [seq, 64] per k-tile via PE transpose, ones column
     appended (softmax denominator rides the ctx matmul for free)
  3. per (batch, q-chunk): scores^T[k, q] for BOTH heads packed into one
     PSUM tile via tile_position row groups (the two K=64 matmuls run
     concurrently on the PE array), one exp per packed tile on ACT,
     ctx^T[d, q] += [v | 1].T @ p^T per head
  4. normalize: DVE reciprocal -> DRAM round-trip broadcast -> DVE multiply
  5. out[q, 1024] = ctx^T.T @ w_o, interleaved into the next attention block

Emission order stages projection groups between attention blocks; Tile's
dependency tracking overlaps them (PE runs proj matmuls whenever attention
stalls on not-yet-projected k-tiles).
"""
import numpy as np

HEADS = 16
DIM_HEAD = 64
QUERY_DIM = 1024
SCALE = DIM_HEAD ** -0.5
B, S = 2, 2048
NSEQ = B * S              # 4096
N_CORES = 8
FEAT = 128                # 2 heads x 64 per core
KT = S // 128             # 16 k-tiles per batch

_nc_cache = None


def _build():
    import concourse.bass as bass
    import concourse.tile as tile
    from concourse import bacc, mybir
    from concourse.masks import make_identity

    F32 = mybir.dt.float32
    F16 = mybir.dt.float16
    AF = mybir.ActivationFunctionType

    nc = bacc.Bacc("TRN2", target_bir_lowering=False)

    hst = nc.dram_tensor("hst", [QUERY_DIM, NSEQ], F16, kind="ExternalInput")
    wq = nc.dram_tensor("wq", [QUERY_DIM, FEAT], F16, kind="ExternalInput")
    wk = nc.dram_tensor("wk", [QUERY_DIM, FEAT], F16, kind="ExternalInput")
    wv = nc.dram_tensor("wv", [QUERY_DIM, FEAT], F16, kind="ExternalInput")
    wo = nc.dram_tensor("wo", [FEAT, QUERY_DIM], F16, kind="ExternalInput")
    out = nc.dram_tensor("out", [NSEQ, QUERY_DIM], F16, kind="ExternalOutput")

    with tile.TileContext(nc) as tc:
        with (
            tc.tile_pool(name="sb", bufs=1) as sb,
            tc.tile_pool(name="ps", bufs=1, space="PSUM") as ps,
            tc.tile_pool(name="dr", bufs=1, space="DRAM") as dr,
        ):
            # ---- constants / weights (hst group 0 is DMA'd first inside
            # proj_group; weight DMAs are emitted there too, in consume
            # order) ----
            wq_sb = sb.tile([128, 8, FEAT], F16)
            wk_sb = sb.tile([128, 8, FEAT], F16)
            wv_sb = sb.tile([128, 8, FEAT], F16)
            wo_sb = sb.tile([128, QUERY_DIM], F16)
            ident = sb.tile([128, 128], F16)

            qT_sb = sb.tile([128, NSEQ], F16)
            kT_sb = sb.tile([128, NSEQ], F16)
            ctxn_sb = sb.tile([128, NSEQ], F16)
            v65 = sb.tile([128, 4 * KT, 65], F16)
            nc.gpsimd.memset(v65[:, :, 64:65], 1.0)
            ones16 = sb.tile([1, 64], F16)
            nc.gpsimd.memset(ones16[:], 1.0)

            hst_src = hst.ap().rearrange("(kt p) n -> p kt n", p=128)
            w_sbs = {"q": wq_sb, "k": wk_sb, "v": wv_sb}

            def proj_one(proj, g, hst_t, ptag="B"):
                """One projection (both 512-halves) for seq-group g."""
                g0 = g * 1024
                vT_t = None
                if proj == "v":
                    vT_t = sb.tile([128, 1024], F16, tag="vT_t", bufs=2,
                                   name=f"vT_t{g}")
                for half in range(2):
                    h0 = half * 512
                    p_p = ps.tile([128, 512], F32, tag=ptag,
                                  bufs=(4 if ptag == "B" else 2),
                                  name=f"p_{proj}{g}_{half}")
                    for kt in range(8):
                        nc.tensor.matmul(
                            p_p[:], w_sbs[proj][:, kt, :],
                            hst_t[:, kt, h0:h0 + 512],
                            start=(kt == 0), stop=(kt == 7),
                        )
                    if proj == "q":
                        nc.vector.tensor_copy(qT_sb[:, g0 + h0:g0 + h0 + 512], p_p[:])
                    elif proj == "k":
                        nc.vector.tensor_copy(kT_sb[:, g0 + h0:g0 + h0 + 512], p_p[:])
                    else:
                        nc.vector.tensor_copy(vT_t[:, h0:h0 + 512], p_p[:])
                if proj == "v":
                    for c in range(8):
                        ci = g * 8 + c
                        b_i, kt_loc = ci // 16, ci % 16
                        p_tr = ps.tile([128, 128], F16, tag=ptag,
                                       bufs=(4 if ptag == "B" else 2),
                                       name=f"p_tr{ci}")
                        nc.tensor.transpose(
                            p_tr[:], vT_t[:, c * 128:(c + 1) * 128], ident[:])
                        # one strided copy fills both heads' v65 slots
                        v65_4d = v65.rearrange("p (pr kt) c -> p pr kt c", pr=4)
                        dst = v65_4d[:, b_i * 2:(b_i + 1) * 2, kt_loc, 0:64]
                        nc.vector.tensor_copy(
                            dst, p_tr.rearrange("p (h d) -> p h d", h=2))

            def load_group(g, chunks=1):
                hst_t = sb.tile([128, 8, 1024], F16, tag="hst_t", bufs=2,
                                name=f"hst_t{g}")
                g0 = g * 1024
                w = 1024 // chunks
                for c in range(chunks):
                    nc.sync.dma_start(hst_t[:, :, c * w:(c + 1) * w],
                                      hst_src[:, :, g0 + c * w:g0 + (c + 1) * w])
                return hst_t

            def proj_group(g, hst_t, ptag="B"):
                for proj in ("k", "q", "v"):
                    proj_one(proj, g, hst_t, ptag)

            def out_qt(qt):
                """One 128-row tile of the final projection."""
                t0 = qt * 128
                o_sb = sb.tile([128, 1024], F16, tag="o_sb", bufs=4,
                               name=f"o_sb{qt}")
                for c in range(2):
                    p_o = ps.tile([128, 512], F32, tag="B", bufs=4,
                                  name=f"p_o{qt}_{c}")
                    nc.tensor.matmul(
                        p_o[:], ctxn_sb[:, t0:t0 + 128],
                        wo_sb[:, c * 512:(c + 1) * 512],
                        start=True, stop=True)
                    nc.vector.tensor_copy(o_sb[:, c * 512:(c + 1) * 512], p_o[:])
                nc.sync.dma_start(out[t0:t0 + 128, :], o_sb[:])

            def attn_part(b_i, cc, kts, p_ctx, out_qts=()):
                """Score+exp+ctx for kt in kts, (batch, 512-q-chunk cc),
                head-packed scores; out_qts interleaved."""
                s0 = b_i * S
                q0 = s0 + cc * 512
                oq = list(out_qts)
                for kt in kts:
                    k0 = s0 + kt * 128
                    p_s = ps.tile([128, 1024], F32, tag="A", bufs=2,
                                  name=f"p_s{b_i}_{cc}_{kt}")
                    # head-packed: head h uses PE row group h*64, writes its
                    # own PSUM bank within the shared tile
                    for h in range(2):
                        hp = slice(h * 64, (h + 1) * 64)
                        nc.tensor.matmul(
                            p_s[:, h * 512:(h + 1) * 512],
                            kT_sb[hp, k0:k0 + 128],
                            qT_sb[hp, q0:q0 + 512],
                            start=True, stop=True,
                            tile_position=(h * 64, 0),
                        )
                    pT = sb.tile([128, 1024], F16, tag="pT", bufs=4,
                                 name=f"pT{b_i}_{cc}_{kt}")
                    nc.scalar.activation(pT[:], p_s[:], AF.Exp)
                    for h in range(2):
                        nc.tensor.matmul(
                            p_ctx[h][:],
                            v65[:, (b_i * 2 + h) * 16 + kt, :],
                            pT[:, h * 512:(h + 1) * 512],
                            start=(kt == 0), stop=(kt == KT - 1),
                        )
                    if kt % 4 == 1 and oq:
                        out_qt(oq.pop(0))
                while oq:
                    out_qt(oq.pop(0))

            def ctx_tiles(b_i, cc):
                return [ps.tile([65, 512], F32, tag="B", bufs=4,
                                name=f"p_ctx{b_i}_{cc}_{h}")
                        for h in range(2)]

            def norm(b_i, cc, p_ctx):
                s0 = b_i * S
                q0 = s0 + cc * 512
                for h in range(2):
                    hp = slice(h * 64, (h + 1) * 64)
                    pc = p_ctx[h]
                    recip = sb.tile([1, 512], F32, tag="recip", bufs=4,
                                    name=f"recip{b_i}_{cc}_{h}")
                    nc.vector.reciprocal(recip[:], pc[64:65, :])
                    r_dr = dr.tile([1, 512], F32, tag="r_dr", bufs=4,
                                   name=f"r_dr{b_i}_{cc}_{h}")
                    nc.sync.dma_start(r_dr[:], recip[:])
                    rbc_sb = sb.tile([64, 512], F32, tag="rbc", bufs=4,
                                     name=f"rbc{b_i}_{cc}_{h}")
                    nc.sync.dma_start(rbc_sb[:],
                                      r_dr[0:1, :].to_broadcast([64, 512]))
                    nc.vector.tensor_mul(
                        ctxn_sb[hp, q0:q0 + 512], pc[0:64, :], rbc_sb[:])

            def attn_block(b_i, cc, out_qts):
                p_ctx = ctx_tiles(b_i, cc)
                attn_part(b_i, cc, range(KT), p_ctx, out_qts)
                norm(b_i, cc, p_ctx)

            # ---- emission order. Program order defines dataflow (producers
            # strictly before consumers); Tile pulls later-emitted
            # independent work (proj for the next batch, out-GEMMs) into
            # engine idle slots. Chunk (0,0)/(0,1) start on g0 data alone
            # (kt 0-7), the rest follows g1. Out-qts lag their chunk by one
            # block ----
            nc.sync.dma_start(wk_sb[:], wk.ap().rearrange("(kt p) m -> p kt m", p=128))
            nc.sync.dma_start(wq_sb[:], wq.ap().rearrange("(kt p) m -> p kt m", p=128))
            hst_g0 = load_group(0, chunks=2)
            nc.sync.dma_start(wv_sb[:], wv.ap().rearrange("(kt p) m -> p kt m", p=128))
            nc.sync.dma_start(wo_sb[:], wo[:])
            make_identity(nc, ident[:])
            proj_one("k", 0, hst_g0)
            proj_one("q", 0, hst_g0)
            proj_one("v", 0, hst_g0)
            ctx00 = ctx_tiles(0, 0)
            attn_part(0, 0, range(0, 8), ctx00)
            proj_group(1, load_group(1))
            # ctx01 allocated only after g1's transient PSUM use drains
            # (ctx00+ctx01 would pin all 4 B-slots and deadlock g1)
            ctx01 = ctx_tiles(0, 1)
            attn_part(0, 1, range(0, 8), ctx01)
            attn_part(0, 0, range(8, KT), ctx00)
            norm(0, 0, ctx00)
            attn_part(0, 1, range(8, KT), ctx01)
            norm(0, 1, ctx01)
            attn_block(0, 2, [0, 1, 2, 3])
            hst_g2 = load_group(2)
            proj_group(2, hst_g2)
            attn_block(0, 3, [4, 5, 6, 7])
            proj_group(3, load_group(3))
            attn_block(1, 0, [8, 9, 10, 11])
            attn_block(1, 1, [12, 13, 14, 15, 16, 17, 18, 19])
            attn_block(1, 2, [20, 21, 22, 23])
            attn_block(1, 3, [24, 25, 26, 27])
            for qt in range(28, 32):
                out_qt(qt)

    nc.finalize()
    return nc


def _get_nc():
    global _nc_cache
    if _nc_cache is None:
        _nc_cache = _build()
    return _nc_cache


def prepare_in_maps(hidden_states, w_q, w_k, w_v, w_o):
    hs = np.asarray(hidden_states, dtype=np.float32).reshape(NSEQ, QUERY_DIM)
    hst = np.ascontiguousarray(hs.T).astype(np.float16)
    wqs = (np.asarray(w_q, dtype=np.float32) * SCALE).astype(np.float16)
    wk16 = np.asarray(w_k, dtype=np.float32).astype(np.float16)
    wv16 = np.asarray(w_v, dtype=np.float32).astype(np.float16)
    wo16 = np.asarray(w_o, dtype=np.float32).astype(np.float16)
    in_maps = []
    for d in range(N_CORES):
        cols = slice(d * FEAT, (d + 1) * FEAT)
        in_maps.append({
            "hst": hst,
            "wq": np.ascontiguousarray(wqs[:, cols]),
            "wk": np.ascontiguousarray(wk16[:, cols]),
            "wv": np.ascontiguousarray(wv16[:, cols]),
            "wo": np.ascontiguousarray(wo16[cols, :]),
        })
    return in_maps


def run_spmd(in_maps, **kwargs):
    from concourse.bass_utils import run_bass_kernel_spmd
    return run_bass_kernel_spmd(_get_nc(), in_maps,
                                core_ids=list(range(N_CORES)), **kwargs)


def kernel(hidden_states, w_q, w_k, w_v, w_o, b_o):
    in_maps = prepare_in_maps(hidden_states, w_q, w_k, w_v, w_o)
    res = run_spmd(in_maps)
    acc = np.zeros((NSEQ, QUERY_DIM), dtype=np.float32)
    for r in res.results:
        acc += r["out"].astype(np.float32)
    acc += np.asarray(b_o, dtype=np.float32)
    return acc.reshape(B, S, QUERY_DIM)
